# revision 14
# baseline (speedup 1.0000x reference)
"""DPConv (kernel=8, ext=4, stride=4) on 8 TRN2 NeuronCores.

Math: with K = k + 2e = 16 and k = 8, every adaptive-pool bin is exactly
2 wide, so the whole DPConv collapses to a separable linear operator:

    out_img = L @ img @ L.T          (per (n, c) image, 128x128)

where L is a 128x128 stencil matrix: for output index w the contributing
windows are i in [max(0,ceil((w-7)/4)), min(30, floor(w/4))] (counted
twice when that range is a single i - the fold count normalization),
each contributing the clamped replicate-padded pair {2w-4i-4, 2w-4i-3}
with weight 1/4 (pool avg 1/2 x fold avg 1/2).

On-chip factorization per image tile [H=128 partitions, W=128 free]:
  rows:  T = (L/4) @ x on TensorE. x is split on the HOST into exact
         bf16 hi/lo halves (x = hi + lo, L itself is exact in bf16's
         8-bit mantissa - its entries are n/16), so two full-rate bf16
         matmuls accumulated in PSUM give ~1e-5 precision at 2x the
         fp32-matmul speed (fp32 matmul = 2 half-rate passes on TRN2).
  cols:  P[v] = T[2v-4]+T[2v-3]  (pairsum: ACT evacuates even cols
         PSUM->SBUF, DVE adds odd cols straight from PSUM)
         out[4a+b] = P[2a+b] + P[2a+b+2]  (fold: single DVE op with an
         as-strided overlapping read AP and a contiguous write - the
         strided-write form runs 2x slower)
  edge columns use x2 scaled copies on ACT / self-adds on DVE.

Sharding: pure data parallel - core k takes batch element n = k.
Host staging per core: transpose to [H, C, W] (so DMA reads are 4 KiB
contiguous per partition) and hi/lo-split to bf16 (same total DMA
bytes as fp32: 2 x 2 MiB in, 4 MiB out per core).
"""

import ml_dtypes
import numpy as np

import concourse.bacc as bacc
import concourse.mybir as mybir
import concourse.tile as tile
from concourse import bass_utils
from concourse.ap import AP

N_CORES = 8
C_PER_CORE = 64          # images per core (= C; one batch element per core)
G = 8                    # images per compute half-group
CHUNK = 4                # images per input DMA chunk
N_GROUPS = C_PER_CORE // G
F32 = mybir.dt.float32
BF16 = mybir.dt.bfloat16
BF16_NP = ml_dtypes.bfloat16


def _build_lq() -> np.ndarray:
    """The 1-D DPConv operator with both 1/4 scalings folded in: L/4."""
    L = np.zeros((128, 128), np.float64)
    for w in range(128):
        i_lo = max(0, -((7 - w) // 4))      # ceil((w-7)/4)
        i_hi = min(30, w // 4)
        for i in (i_lo, i_hi):              # counted twice when equal
            L[w, min(127, max(0, 2 * w - 4 * i - 4))] += 0.25
            L[w, min(127, max(0, 2 * w - 4 * i - 3))] += 0.25
    return (L / 4.0).astype(np.float32)


_LQ_T = np.ascontiguousarray(_build_lq().T)          # lhsT layout [r, h]
_LQ_T_BF16 = _LQ_T.astype(BF16_NP)
assert np.all(_LQ_T_BF16.astype(np.float32) == _LQ_T)  # L exact in bf16


def _as_strided(base: AP, dims) -> AP:
    """Rebuild `base` (a sliced AP pointing at the wanted offset) with
    explicit [stride, size] free dims (overlapping reads allowed)."""
    return AP(base.tensor, base.offset, dims)


def _dpconv_tile(tc, o_d, xhl_d, lt_d):
    nc = tc.nc
    with tc.tile_pool(name="const", bufs=1) as cp, \
         tc.tile_pool(name="in", bufs=12) as inp, \
         tc.tile_pool(name="io", bufs=6) as iop, \
         tc.tile_pool(name="mid", bufs=5) as mp, \
         tc.tile_pool(name="ps", bufs=4, space="PSUM") as pp:
        lt = cp.tile([128, 128], BF16)
        nc.sync.dma_start(out=lt[:], in_=lt_d)
        for g in range(N_GROUPS):
            # input arrives in 4-image 256 KiB chunks (sync/SP HWDGE
            # ring, FIFO) so the first matmul starts ~3 us earlier and
            # prefetch is fine-grained; each chunk feeds one mm pair.
            chunks = []
            for h in range(2):
                sl4 = slice(g * G + CHUNK * h, g * G + CHUNK * (h + 1))
                ct = inp.tile([128, 2, CHUNK, 128], BF16, tag="in")
                nc.sync.dma_start(
                    out=ct[:],
                    in_=xhl_d[:, :, sl4, :].rearrange("s h c w -> h s c w"))
                chunks.append(ct)

            # rows: T = (L/4) @ (hi + lo), PSUM-accumulated per 512-chunk
            t1 = pp.tile([128, G, 128], F32, tag="t1")
            for h, ct in enumerate(chunks):
                cs = slice(CHUNK * h, CHUNK * (h + 1))
                nc.tensor.matmul(t1[:, cs, :], lt[:], ct[:, 0],
                                 start=True, stop=False)
                nc.tensor.matmul(t1[:, cs, :], lt[:], ct[:, 1],
                                 start=False, stop=True)

            # cols step 1: pairsum P[v] = T[2v-4] + T[2v-3] (clamped).
            # TensorTensor may read at most ONE input from PSUM: ACT
            # (close to PSUM, otherwise idle) evacuates the even cols,
            # DVE adds the odd cols from PSUM. Edge cols are x2 scaled
            # broadcast copies on ACT.
            pe_t = mp.tile([128, G, 64], F32, tag="pe")
            nc.scalar.copy(out=pe_t[:], in_=t1[:, :, 0:128:2])
            pt = mp.tile([128, G, 68], F32, tag="P")
            gdim = list(pt[:].ap[1])            # [68-ish pitch, G]
            pdim0 = list(pt[:].ap[0])           # partition dim
            tdim = t1[:].ap
            nc.vector.tensor_add(
                out=pt[:, :, 2:66], in0=pe_t[:], in1=t1[:, :, 1:128:2])
            # P edge cols {0,1,66,67} = 2x T cols {0,0,127,127}: one ACT
            # op - out strides (66,1), in strides (127, 0-broadcast)
            nc.scalar.mul(
                _as_strided(pt[:, :, 0:1], [pdim0, gdim, [66, 2], [1, 2]]),
                _as_strided(t1[:, :, 0:1],
                            [list(tdim[0]), list(tdim[1]), [127, 2], [0, 2]]),
                2.0)

            # cols step 2: fold out[4a+b] = P[2a+b] + P[2a+b+2], with
            # overlapping as-strided reads (a:30 x2, b:4 x1) and a
            # contiguous write of cols 4..123 - split half/half between
            # GpSimd and DVE (both otherwise under-used) so neither
            # paces the chain. Edge cols {0..3,124..127} =
            # 2x P{0..3,64..67} ride ACT as a scaled two-region copy.
            ot = iop.tile([128, G, 128], F32, tag="out")
            odim = ot[:].ap
            in0 = _as_strided(pt[:, :, 2:3], [pdim0, gdim, [2, 30], [1, 4]])
            in1 = _as_strided(pt[:, :, 4:5], [pdim0, gdim, [2, 30], [1, 4]])
            out_f = _as_strided(
                ot[:, :, 4:5], [list(odim[0]), list(odim[1]), [4, 30], [1, 4]])
            nc.gpsimd.tensor_add(out=out_f, in0=in0, in1=in1)
            edge_in = _as_strided(pt[:, :, 0:1], [pdim0, gdim, [64, 2], [1, 4]])
            edge_out = _as_strided(
                ot[:, :, 0:1], [list(odim[0]), list(odim[1]), [124, 2], [1, 4]])
            nc.scalar.mul(edge_out, edge_in, 2.0)

            # stores ride the ACT HWDGE ring so they never FIFO behind
            # upcoming loads on the SP ring
            nc.scalar.dma_start(
                out=o_d[slice(g * G, (g + 1) * G)].rearrange("c h w -> h c w"),
                in_=ot[:])


_CACHE = {}


def _get_nc():
    if "nc" not in _CACHE:
        nc = bacc.Bacc("TRN2", target_bir_lowering=False, debug=False)
        xhl_d = nc.dram_tensor("xhl", (2, 128, C_PER_CORE, 128), BF16,
                               kind="ExternalInput").ap()
        lt_d = nc.dram_tensor("lt", (128, 128), BF16,
                              kind="ExternalInput").ap()
        o_d = nc.dram_tensor("o", (C_PER_CORE, 128, 128), F32,
                             kind="ExternalOutput").ap()
        with tile.TileContext(nc) as tc:
            _dpconv_tile(tc, o_d, xhl_d, lt_d)
        nc.compile()
        _CACHE["nc"] = nc
    return _CACHE["nc"]


def _stage(xk: np.ndarray) -> np.ndarray:
    """[C,H,W] f32 -> [2,H,C,W] bf16 (exact hi/lo split, H-major)."""
    xt = np.ascontiguousarray(xk.transpose(1, 0, 2))          # [H,C,W]
    hi = xt.astype(BF16_NP)
    lo = (xt - hi.astype(np.float32)).astype(BF16_NP)
    return np.stack([hi, lo], axis=0)


def run(x: np.ndarray, **spmd_kwargs) -> bass_utils.BassKernelResults:
    """Shard x (8,64,128,128) across 8 cores and run the Bass kernel."""
    nc = _get_nc()
    in_maps = [
        {"xhl": _stage(x[k]), "lt": _LQ_T_BF16} for k in range(N_CORES)
    ]
    return bass_utils.run_bass_kernel_spmd(
        nc, in_maps, core_ids=list(range(N_CORES)), **spmd_kwargs)


def kernel(x) -> np.ndarray:
    x = np.asarray(x, dtype=np.float32)
    assert x.shape == (N_CORES, C_PER_CORE, 128, 128), x.shape
    res = run(x)
    return np.stack([res.results[k]["o"] for k in range(N_CORES)], axis=0)
